# revision 22
# baseline (speedup 1.0000x reference)
"""Trainium2 Bass kernel for the JointLoss problem (contrastive NT-Xent + 2 MSE terms).

kernel(representation, xrecon, xorig) -> (loss, closs, recon_loss, zrecon_loss)

Strategy (8 NeuronCores, SPMD — one NEFF, per-core variation only via inputs):
  - Host normalizes the representations (the sharding hint's "all-gather of the
    normalized representations" — normalization happens before the gather in the
    data-parallel formulation), scales by S=16 so fp8 e4m3 stays in its normal
    range, and ships z^T slabs per core.  sim/tau is recovered by folding
    1/(S^2 tau) into the exp's constant scale.
  - Each core computes a (512, 2560) slab of q = (S z_i)·(S z_j) against column
    chunks [partner, own, +1, +2, +3] using fp8 DoubleRow matmuls (K=256 per
    instruction).  exp runs on Act with per-call row-sum accumulators; chunks
    +1..+3 write fp8 exp tiles whose column sums (one DoubleRow ones-matmul
    pair per chunk) supply the transposed contributions to other cores' rows.
  - Positives come from the diagonal of the partner block, extracted from PSUM
    with an identity mask multiply + free-axis reduce on DVE.  (The native
    tensor_tensor_reduce ISA op crashes the exec unit on this platform.)
  - MSE terms: bf16 subtract + square on DVE; the xrecon sum-of-squares
    reduces via accumulating ones-matmul column sums on PE (gpsimd XYZWC
    reduce is a slow software loop on real hardware), the small zi/zj sum
    via a per-partition DVE reduce summed on the host.
  - Host combine: sum the 8 cores' partial row sums + pushed column sums,
    subtract exp(1/tau) for the self column, log, and the two MSE scalars.
"""

import math

import ml_dtypes
import numpy as np

TAU = 0.5
EPS = 1e-8
N = 2048
TWO_N = 4096
D = 512
NCORES = 8
CH = 512
S = 16.0  # fp8 pre-scale for normalized vectors
QS = 1.0 / (S * S * TAU)  # exp input scale

_CACHE = {}


def _build_nc():
    import concourse.bacc as bacc
    import concourse.mybir as mybir
    import concourse.tile as tile
    from concourse.masks import make_identity

    F32 = mybir.dt.float32
    BF16 = mybir.dt.bfloat16
    FP8 = mybir.dt.float8e4
    OP = mybir.AluOpType
    AF = mybir.ActivationFunctionType
    AX = mybir.AxisListType
    DR = mybir.MatmulPerfMode.DoubleRow

    nc = bacc.Bacc("TRN2", target_bir_lowering=False, debug=False)
    # zt[ch][p][d*512+c] = (S*z)^T[d*128+p, 512*ch + c] (permuted cols, 5 chunks)
    zt = nc.dram_tensor("zt", [5, 128, 2048], FP8, kind="ExternalInput")
    xr = nc.dram_tensor("xr", [128, 4096], BF16, kind="ExternalInput")
    xo = nc.dram_tensor("xo", [128, 4096], BF16, kind="ExternalInput")
    zi = nc.dram_tensor("zi", [128, 1024], BF16, kind="ExternalInput")
    zj = nc.dram_tensor("zj", [128, 1024], BF16, kind="ExternalInput")
    out1 = nc.dram_tensor("out1", [128, 20], F32, kind="ExternalOutput")
    ocol = nc.dram_tensor("ocol", [1, 2048], F32, kind="ExternalOutput")

    with tile.TileContext(nc) as tc:
        with (
            tc.tile_pool(name="singles", bufs=1) as singles,
            tc.tile_pool(name="ebap", bufs=2) as ebap,
            tc.tile_pool(name="msep", bufs=2) as msep,
            tc.tile_pool(name="mpsum", bufs=2, space="PSUM") as mpsum,
            tc.tile_pool(name="capsum", bufs=1, space="PSUM") as capsum,
            tc.tile_pool(name="qpsum", bufs=1, space="PSUM") as qpsum,
        ):
            identf = singles.tile([128, 128], F32, tag="identf")
            make_identity(nc, identf)
            # dual-fp8 ldweights requires the k-pair stride in the weights AP
            # to be 16-byte aligned (walrus s3_lw_dual_fp8_restrictions), so
            # the ones live in a [128, 2, 16] tile sliced to [:, :, 0:2]
            ones8 = singles.tile([128, 2, 16], FP8, tag="ones8")
            nc.vector.memset(ones8, 1.0)
            # acc columns: 0-11 eacc[rr*3+blk], 12-15 pos[rr], 16-17 dx halves,
            # 18 dz, 19 unused
            acc = singles.tile([128, 20], F32, tag="acc")
            eb = singles.tile([128, 4, 1536], FP8, tag="eb")

            # input tiles + DMA (zt chunks 0/1 split at the d=2 boundary so the
            # dd0 matmuls can start as soon as the first half lands)
            zt_t = []
            for ch in range(5):
                t = singles.tile([128, 4, 512], FP8, tag=f"zt_{ch}")
                if ch < 2:
                    nc.sync.dma_start(t[:, 0:2, :], zt[ch][:, 0:1024])
                    nc.sync.dma_start(t[:, 2:4, :], zt[ch][:, 1024:2048])
                else:
                    nc.sync.dma_start(t, zt[ch])
                zt_t.append(t)

            zit = singles.tile([128, 1024], BF16, tag="zit")
            nc.sync.dma_start(zit, zi[:, :])
            zjt = singles.tile([128, 1024], BF16, tag="zjt")
            nc.sync.dma_start(zjt, zj[:, :])
            xrt = singles.tile([128, 4096], BF16, tag="xrt")
            xot = singles.tile([128, 4096], BF16, tag="xot")
            nc.sync.dma_start(xrt[:, 0:2048], xr[:, 0:2048])
            nc.sync.dma_start(xot[:, 0:2048], xo[:, 0:2048])
            nc.sync.dma_start(xrt[:, 2048:4096], xr[:, 2048:4096])
            nc.sync.dma_start(xot[:, 2048:4096], xo[:, 2048:4096])

            for rr in range(4):
                psA = mpsum.tile([128, 1024], F32, tag="ps")
                psB = mpsum.tile([128, 1024], F32, tag="ps")
                psC = mpsum.tile([128, 512], F32, tag="ps")
                for dd in range(2):
                    w = zt_t[1][:, 2 * dd : 2 * dd + 2, 128 * rr : 128 * (rr + 1)]
                    for ch in range(5):
                        if ch < 2:
                            dst = psA[:, CH * ch : CH * (ch + 1)]
                        elif ch < 4:
                            dst = psB[:, CH * (ch - 2) : CH * (ch - 1)]
                        else:
                            dst = psC
                        nc.tensor.matmul(
                            dst,
                            w,
                            zt_t[ch][:, 2 * dd : 2 * dd + 2, :],
                            start=(dd == 0),
                            stop=(dd == 1),
                            perf_mode=DR,
                        )
                # positives: diagonal of the partner block (raw q, pre-exp)
                ext = msep.tile([128, 128], F32, tag="ext")
                nc.vector.tensor_tensor(
                    ext, psA[:, 128 * rr : 128 * (rr + 1)], identf, OP.mult
                )
                nc.vector.reduce_sum(acc[:, 12 + rr : 13 + rr], ext, axis=AX.X)
                ebA = ebap.tile([128, 1024], FP8, tag="ebA")
                nc.scalar.activation(
                    ebA, psA, AF.Exp, scale=QS, accum_out=acc[:, 3 * rr : 3 * rr + 1]
                )
                nc.scalar.activation(
                    eb[:, rr, 0:1024],
                    psB,
                    AF.Exp,
                    scale=QS,
                    accum_out=acc[:, 3 * rr + 1 : 3 * rr + 2],
                )
                nc.scalar.activation(
                    eb[:, rr, 1024:1536],
                    psC,
                    AF.Exp,
                    scale=QS,
                    accum_out=acc[:, 3 * rr + 2 : 3 * rr + 3],
                )

            # column sums of chunks +1/+2/+3 exp tiles (DoubleRow over rr pairs),
            # staged PSUM -> SBUF per segment (PSUM is not DMA-able)
            cap = capsum.tile([2, 1536], F32, tag="cap")
            ocolt = singles.tile([1, 2048], F32, tag="ocolt")
            for ch in range(3):
                for j in range(2):
                    nc.tensor.matmul(
                        cap[0:2, CH * ch : CH * (ch + 1)],
                        ones8[:, :, 0:2],
                        eb[:, 2 * j : 2 * j + 2, CH * ch : CH * (ch + 1)],
                        start=(j == 0),
                        stop=(j == 1),
                        perf_mode=DR,
                    )
                nc.vector.tensor_copy(
                    ocolt[0:1, CH * ch : CH * (ch + 1)],
                    cap[0:1, CH * ch : CH * (ch + 1)],
                )

            # MSE partials: bf16 subtract + square on DVE; dx sum via
            # accumulating ones-matmul column sums into one PSUM bank, dz sum
            # via per-partition DVE reduce (host adds the 512+128 partials)
            ones_bf = singles.tile([128, 1], BF16, tag="ones_bf")
            nc.vector.memset(ones_bf, 1.0)
            dxs = singles.tile([128, 4096], BF16, tag="dxs")
            dzs = singles.tile([128, 1024], BF16, tag="dzs")

            def mse(a, b, sq):
                w = a.shape[-1]
                d = msep.tile([128, 2048], BF16, tag="d")
                nc.vector.tensor_tensor(d[:, 0:w], a, b, OP.subtract)
                nc.vector.tensor_tensor(sq, d[:, 0:w], d[:, 0:w], OP.mult)

            mse(zit, zjt, dzs)
            nc.vector.reduce_sum(acc[:, 16:17], dzs, axis=AX.X)
            mse(xrt[:, 0:2048], xot[:, 0:2048], dxs[:, 0:2048])
            mse(xrt[:, 2048:4096], xot[:, 2048:4096], dxs[:, 2048:4096])
            qcap = qpsum.tile([1, 512], F32, tag="qcap")
            for blk in range(8):
                nc.tensor.matmul(
                    qcap,
                    ones_bf,
                    dxs[:, CH * blk : CH * (blk + 1)],
                    start=(blk == 0),
                    stop=(blk == 7),
                )
            nc.vector.tensor_copy(ocolt[0:1, 1536:2048], qcap)

            nc.sync.dma_start(out1[:, :], acc)
            nc.sync.dma_start(ocol[:, :], ocolt)

    nc.compile()
    return nc


def _get_nc():
    if "nc" not in _CACHE:
        _CACHE["nc"] = _build_nc()
    return _CACHE["nc"]


def make_in_maps(representation, xrecon, xorig):
    rep = np.ascontiguousarray(np.asarray(representation, dtype=np.float32))
    nrm = np.maximum(np.linalg.norm(rep, axis=1, keepdims=True), EPS)
    u = (rep / nrm) * S
    uq = u.astype(ml_dtypes.float8_e4m3)
    UT = np.ascontiguousarray(uq.T)  # (512, 4096) fp8
    xrec = np.asarray(xrecon, dtype=np.float32).astype(ml_dtypes.bfloat16)
    xorg = np.asarray(xorig, dtype=np.float32).astype(ml_dtypes.bfloat16)
    repb = rep.astype(ml_dtypes.bfloat16)
    in_maps = []
    for c in range(NCORES):
        partner = (c + 4) % 8
        order = [partner, c, (c + 1) % 8, (c + 2) % 8, (c + 3) % 8]
        ut_c = np.concatenate([UT[:, CH * p : CH * (p + 1)] for p in order], axis=1)
        # [d, p, ch, col] -> [ch, p, d, col]  (ch = 512-col chunk index)
        zt_c = np.ascontiguousarray(
            ut_c.reshape(4, 128, 5, 512).transpose(2, 1, 0, 3).reshape(5, 128, 2048)
        )
        in_maps.append(
            {
                "zt": zt_c,
                "xr": np.ascontiguousarray(
                    xrec[CH * c : CH * (c + 1)]
                    .reshape(4, 128, 1024).transpose(1, 0, 2).reshape(128, 4096)
                ),
                "xo": np.ascontiguousarray(
                    xorg[CH * c : CH * (c + 1)]
                    .reshape(4, 128, 1024).transpose(1, 0, 2).reshape(128, 4096)
                ),
                "zi": np.ascontiguousarray(
                    repb[256 * c : 256 * (c + 1)]
                    .reshape(2, 128, D).transpose(1, 0, 2).reshape(128, 1024)
                ),
                "zj": np.ascontiguousarray(
                    repb[2048 + 256 * c : 2048 + 256 * (c + 1)]
                    .reshape(2, 128, D).transpose(1, 0, 2).reshape(128, 1024)
                ),
            }
        )
    return in_maps


def combine_outputs(results):
    """results: list of 8 dicts with out1 [128,20], ocol [1,2048]."""
    E2 = math.exp(1.0 / TAU)
    denom = np.zeros(TWO_N, dtype=np.float64)
    pos = np.zeros(TWO_N, dtype=np.float64)
    dxs = 0.0
    dzs = 0.0
    for c in range(NCORES):
        a = np.asarray(results[c]["out1"], dtype=np.float64)  # [128, 20]
        # partition p, row group rr -> global row 512c + 128rr + p
        rsum = a[:, 0:12].reshape(128, 4, 3).sum(axis=2)  # [p, rr]
        denom[CH * c : CH * (c + 1)] += rsum.T.reshape(-1)
        pos[CH * c : CH * (c + 1)] = a[:, 12:16].T.reshape(-1)
        oc = np.asarray(results[c]["ocol"], dtype=np.float64).reshape(-1)
        for k in range(3):
            m = (c + 1 + k) % NCORES
            denom[CH * m : CH * (m + 1)] += oc[CH * k : CH * (k + 1)]
        dxs += oc[1536:2048].sum()
        dzs += a[:, 16].sum()
    denom -= E2
    closs = (np.log(denom) - pos * QS).sum() / TWO_N
    recon = dxs / TWO_N
    zrec = dzs / N
    loss = recon + closs + zrec
    f = np.float32
    return (f(loss), f(closs), f(recon), f(zrec))


def kernel(representation, xrecon, xorig):
    from concourse.bass_utils import run_bass_kernel_spmd

    nc = _get_nc()
    in_maps = make_in_maps(representation, xrecon, xorig)
    res = run_bass_kernel_spmd(nc, in_maps, core_ids=list(range(NCORES)))
    return combine_outputs(res.results)


# revision 50
# speedup vs baseline: 3.5336x; 3.5336x over previous
"""Trainium2 Bass kernel for the JointLoss problem (contrastive NT-Xent + 2 MSE terms).

kernel(representation, xrecon, xorig) -> (loss, closs, recon_loss, zrecon_loss)

Strategy (8 NeuronCores, SPMD — one NEFF, per-core variation only via inputs):
  - Host normalizes the representations (the sharding hint's "all-gather of the
    normalized representations" — normalization happens before the gather in the
    data-parallel formulation), scales by S=16 so fp8 e4m3 stays in its normal
    range, and ships z^T slabs per core.  sim/tau is recovered by folding
    1/(S^2 tau) into the exp's constant scale.
  - Each core computes a (512, 2560) slab of q = (S z_i)·(S z_j) against column
    chunks [partner, own, +1, +2, +3] using fp8 DoubleRow matmuls (K=256 per
    instruction).  exp runs on Act with per-call row-sum accumulators; chunks
    +1..+3 write fp8 exp tiles whose column sums (one DoubleRow ones-matmul
    per rr pair) supply the transposed contributions to other cores' rows.
  - Positives come from the diagonal of the partner block, extracted from PSUM
    with an identity mask multiply + free-axis reduce on DVE.  (The native
    tensor_tensor_reduce ISA op crashes the exec unit on this platform.)
  - MSE terms: bf16 subtract + square on DVE over interleaved xr|xo quarters
    as they land; sums via accumulating ones-matmul column sums on PE (dx)
    and a per-partition DVE reduce (dz); host adds the partials.
  - Inputs are packed into 3 DRAM tensors and streamed as 10 large DMAs
    (the hardware DGE has a ~0.6us per-transfer floor): the dd0 halves of
    the first two chunks land first so the GEMM starts at ~3us.
  - Host combine: sum the 8 cores' partial row sums + pushed column sums,
    subtract exp(1/tau) for the self column, log, and the two MSE scalars.
"""

import math

import ml_dtypes
import numpy as np

TAU = 0.5
EPS = 1e-8
N = 2048
TWO_N = 4096
D = 512
NCORES = 8
CH = 512
S = 16.0  # fp8 pre-scale for normalized vectors
QS = 1.0 / (S * S * TAU)  # exp input scale

_CACHE = {}


def _build_nc():
    import concourse.bacc as bacc
    import concourse.mybir as mybir
    import concourse.tile as tile

    F32 = mybir.dt.float32
    BF16 = mybir.dt.bfloat16
    FP8 = mybir.dt.float8e4
    OP = mybir.AluOpType
    AF = mybir.ActivationFunctionType
    AX = mybir.AxisListType
    DR = mybir.MatmulPerfMode.DoubleRow

    nc = bacc.Bacc("TRN2", target_bir_lowering=False, debug=False)
    # ztp per-partition layout (fp8 bytes):
    #   [ch0dd0(1024) | ch1dd0(1024) | ch0dd1(1024) | ch1dd1(1024)
    #    | ch2(2048) | ch3(2048) | ch4(2048)]
    # where chKddJ = [d-pair(2) x 512 cols] and ch2/3/4 = [d(4) x 512 cols];
    # ztp[p] holds z^T rows {d*128+p}. Chunk order: partner, own, +1, +2, +3.
    ztp = nc.dram_tensor("ztp", [128, 10240], FP8, kind="ExternalInput")
    # X: [q(4) x io(2) x 1024] bf16 per partition — xrecon/xorig quarters
    xin = nc.dram_tensor("xin", [128, 8192], BF16, kind="ExternalInput")
    # Z: [io(2) x 1024] bf16 — zi | zj
    zin = nc.dram_tensor("zin", [128, 2048], BF16, kind="ExternalInput")
    # 128x128 identity from the host (gpsimd affine_select would drag the
    # Pool engine into the final drain barrier)
    idf = nc.dram_tensor("idf", [128, 128], F32, kind="ExternalInput")
    out1a = nc.dram_tensor("out1a", [128, 12], F32, kind="ExternalOutput")
    out1b = nc.dram_tensor("out1b", [128, 18], F32, kind="ExternalOutput")

    with tile.TileContext(nc) as tc:
        with (
            tc.tile_pool(name="singles", bufs=1) as singles,
            tc.tile_pool(name="msep", bufs=2) as msep,
            tc.tile_pool(name="extp", bufs=2) as extp,
            tc.tile_pool(name="mpsum", bufs=2, space="PSUM") as mpsum,
            tc.tile_pool(name="cpsum", bufs=1, space="PSUM") as cpsum,
            tc.tile_pool(name="qpsum", bufs=1, space="PSUM") as qpsum,
            tc.tile_pool(name="qxpsum", bufs=1, space="PSUM") as qxpsum,
        ):
            identf = singles.tile([128, 128], F32, tag="identf")
            # dual-fp8 ldweights requires the k-pair stride in the weights AP
            # to be 16-byte aligned (walrus s3_lw_dual_fp8_restrictions), so
            # the ones live in a [128, 2, 16] tile sliced to [:, :, 0:2]
            ones8 = singles.tile([128, 2, 16], FP8, tag="ones8")
            nc.vector.memset(ones8, 1.0)
            ones_bf = singles.tile([128, 1], BF16, tag="ones_bf")
            nc.vector.memset(ones_bf, 1.0)
            # accA: 0-11 eacc[rr*3+blk] (Act accumulators); accD: 0-3 pos[rr],
            # 4 dz partials (DVE) — separate tiles so the tile-granular dep
            # tracker doesn't serialize Act and DVE against each other
            accA = singles.tile([128, 12], F32, tag="accA")
            # accD: 0-3 pos[rr], 4 dz partials, 5-16 exp column-sum blocks,
            # 17 dx partials
            accD = singles.tile([128, 18], F32, tag="accD")
            ebJ = []
            for j in range(2):
                t = singles.tile([128, 2, 1536], FP8, tag=f"ebJ_{j}")
                ebJ.append(t)

            ebA_t = []
            for i in range(2):
                t = singles.tile([128, 1024], FP8, tag=f"ebA_{i}")
                ebA_t.append(t)

            # explicit zero bias AP for Exp: the float-bias path materializes
            # a const tensor whose DMA lands at the head of the input stream,
            # delaying every zt transfer by one slot
            zbias = singles.tile([128, 1], F32, tag="zbias")
            nc.vector.memset(zbias, 0.0)

            # act-table warmup: a no-dep Exp at t~0 so LoadActFuncSet isn't
            # gated behind the first PSUM tile
            warm = singles.tile([128, 1], F32, tag="warm")
            nc.vector.memset(warm, 0.0)
            nc.scalar.activation(warm, warm, AF.Exp, bias=zbias)

            # --- input DMA stream (all on the SP hardware DGE) ---
            ztAB = []  # [dd] -> [128, ch(2), d2(2), 512]
            for dd in range(2):
                t = singles.tile([128, 2, 2, 512], FP8, tag=f"ztAB_{dd}")
                ztAB.append(t)
            zt_t = {}
            for ch in (2, 3, 4):
                t = singles.tile([128, 4, 512], FP8, tag=f"zt_{ch}")
                zt_t[ch] = t
            nc.sync.dma_start(ztAB[0], ztp[:, 0:2048])
            nc.sync.dma_start(ztAB[1], ztp[:, 2048:4096])
            for k, ch in enumerate((2, 3, 4)):
                nc.sync.dma_start(
                    zt_t[ch], ztp[:, 4096 + 2048 * k : 4096 + 2048 * (k + 1)]
                )
            nc.sync.dma_start(identf, idf[:, :])
            zq = singles.tile([128, 2, 1024], BF16, tag="zq")
            nc.sync.dma_start(zq, zin[:, :])
            xq = []
            for k in range(4):
                t = singles.tile([128, 2, 1024], BF16, tag=f"xq_{k}")
                xq.append(t)
                nc.sync.dma_start(t, xin[:, 2048 * k : 2048 * (k + 1)])

            qqj = []
            for j in range(2):
                t = qpsum.tile([128, 12], F32, tag=f"qq_{j}")
                qqj.append(t)
            qdx = qxpsum.tile([128, 1], F32, tag="qdx")
            dxq_t = []
            for k in range(4):
                t = singles.tile([128, 1024], BF16, tag=f"dxq_{k}")
                dxq_t.append(t)
            dzs = singles.tile([128, 1024], BF16, tag="dzs")

            def colsum_wave(j):
                # column sums of chunks +1/+2/+3 exp tiles over one rr pair,
                # partition-major: transpose-reduce (exp tile as weights, ones
                # moving) gives [128,1] per 128-column block
                # one accumulation group per PSUM bank (start marks the whole
                # bank's zero region): each wave owns a bank, summed on DVE
                for ch in range(3):
                    for blk in range(4):
                        nc.tensor.matmul(
                            qqj[j][:, 4 * ch + blk : 4 * ch + blk + 1],
                            ebJ[j][:, :, CH * ch + 128 * blk : CH * ch + 128 * (blk + 1)],
                            ones8[:, :, 0:1],
                            start=True,
                            stop=True,
                            perf_mode=DR,
                        )

            def mm_block(rr, chunks, dst_of):
                for dd in range(2):
                    w = ztAB[dd][:, 1, :, 128 * rr : 128 * (rr + 1)]
                    for ch in chunks:
                        rhs = ztAB[dd][:, ch] if ch < 2 else zt_t[ch][:, 2 * dd : 2 * dd + 2, :]
                        nc.tensor.matmul(
                            dst_of(ch),
                            w,
                            rhs,
                            start=(dd == 0),
                            stop=(dd == 1),
                            perf_mode=DR,
                        )

            def exp_to(dst, src, col):
                nc.scalar.activation(
                    dst,
                    src,
                    AF.Exp,
                    bias=zbias,
                    scale=QS,
                    accum_out=accA[:, col : col + 1],
                )

            def pos_extract(rr, psA):
                # positives: diagonal of the partner block (raw q, pre-exp)
                ext = extp.tile([128, 128], F32, tag="ext")
                nc.vector.tensor_tensor(
                    ext, psA[:, 128 * rr : 128 * (rr + 1)], identf, OP.mult
                )
                nc.vector.reduce_sum(accD[:, rr : rr + 1], ext, axis=AX.X)

            def mse_sub_sq(src, sq):
                d = msep.tile([128, 1024], BF16, tag="d")
                nc.vector.tensor_tensor(d, src[:, 0], src[:, 1], OP.subtract)
                nc.vector.tensor_tensor(sq, d, d, OP.mult)

            def qcap_mm(k):
                # per-128-col-block transpose-reduce of dx^2, all blocks
                # accumulated into one [128,1] column
                for blk in range(8):
                    nc.tensor.matmul(
                        qdx,
                        dxq_t[k][:, 128 * blk : 128 * (blk + 1)],
                        ones_bf,
                        start=(k == 0 and blk == 0),
                        stop=(k == 3 and blk == 7),
                    )

            for rr in range(4):
                psA = mpsum.tile([128, 1024], F32, tag="ps")
                psB = mpsum.tile([128, 1024], F32, tag="ps")
                psC = cpsum.tile([128, 512], F32, tag="psC")
                dstA = lambda ch: psA[:, CH * ch : CH * (ch + 1)]
                dstBC = lambda ch: psB[:, CH * (ch - 2) : CH * (ch - 1)] if ch < 4 else psC

                if rr < 3:
                    mm_block(rr, (0, 1), dstA)
                    mm_block(rr, (2, 3, 4), dstBC)
                else:
                    # rr3: colsum-feeding blocks B/C first so the last colsum
                    # wave and PSUM->SBUF copies overlap with exp A(3)
                    mm_block(rr, (2, 3, 4), dstBC)
                    # fill the PE wait for the psum ring with ready dx sums
                    qcap_mm(0)
                    qcap_mm(1)
                    mm_block(rr, (0, 1), dstA)

                # interleave MSE work into the DVE queue by DMA readiness
                if rr == 1:
                    mse_sub_sq(zq, dzs)
                    nc.vector.reduce_sum(accD[:, 4:5], dzs, axis=AX.X)
                    mse_sub_sq(xq[0], dxq_t[0])
                elif rr == 2:
                    mse_sub_sq(xq[1], dxq_t[1])
                    mse_sub_sq(xq[2], dxq_t[2])
                elif rr == 3:
                    mse_sub_sq(xq[3], dxq_t[3])

                ebA = ebA_t[rr % 2]
                ebr = ebJ[rr // 2][:, rr % 2]
                if rr < 3:
                    exp_to(ebA, psA, 3 * rr)
                    exp_to(ebr[:, 0:1024], psB, 3 * rr + 1)
                    exp_to(ebr[:, 1024:1536], psC, 3 * rr + 2)
                    # pos AFTER the exps: PSUM readers are serialized in
                    # emission order, a DVE reader first would gate exp A
                    pos_extract(rr, psA)
                else:
                    exp_to(ebr[:, 0:1024], psB, 3 * rr + 1)
                    exp_to(ebr[:, 1024:1536], psC, 3 * rr + 2)
                    colsum_wave(1)
                    exp_to(ebA, psA, 3 * rr)
                    pos_extract(rr, psA)
                if rr == 1:
                    colsum_wave(0)

            qcap_mm(2)
            qcap_mm(3)
            nc.vector.tensor_copy(accD[:, 5:17], qqj[0])
            nc.vector.tensor_tensor(accD[:, 5:17], accD[:, 5:17], qqj[1], OP.add)
            nc.vector.tensor_copy(accD[:, 17:18], qdx)

            nc.sync.dma_start(out1b[:, :], accD)
            nc.sync.dma_start(out1a[:, :], accA)

    # Force a single activation-function table: Exp and Copy both live in the
    # natural_log_exp_and_others set, but the load-insertion pass greedily
    # picks the first set per function and would reload between them.
    import concourse.bacc as bacc_mod
    from concourse.hw_specs import get_activation_tables

    real = get_activation_tables(nc.m.arch)
    target = "natural_log_exp_and_others"
    assert target in real
    filtered = {k: (v if k == target else set()) for k, v in real.items()}
    orig = bacc_mod.get_activation_tables
    bacc_mod.get_activation_tables = lambda arch: filtered
    try:
        nc.compile()
    finally:
        bacc_mod.get_activation_tables = orig
    return nc


def _get_nc():
    if "nc" not in _CACHE:
        _CACHE["nc"] = _build_nc()
    return _CACHE["nc"]


def make_in_maps(representation, xrecon, xorig):
    rep = np.ascontiguousarray(np.asarray(representation, dtype=np.float32))
    nrm = np.maximum(np.linalg.norm(rep, axis=1, keepdims=True), EPS)
    u = (rep / nrm) * S
    uq = u.astype(ml_dtypes.float8_e4m3)
    UT = np.ascontiguousarray(uq.T)  # (512, 4096) fp8
    xrec = np.asarray(xrecon, dtype=np.float32).astype(ml_dtypes.bfloat16)
    xorg = np.asarray(xorig, dtype=np.float32).astype(ml_dtypes.bfloat16)
    repb = rep.astype(ml_dtypes.bfloat16)
    in_maps = []
    for c in range(NCORES):
        partner = (c + 4) % 8
        order = [partner, c, (c + 1) % 8, (c + 2) % 8, (c + 3) % 8]
        ut_c = np.concatenate([UT[:, CH * p : CH * (p + 1)] for p in order], axis=1)
        # (512, 2560) -> [ch, p, d, col]: zt[ch][p][d*512+c]
        zt_c = ut_c.reshape(4, 128, 5, 512).transpose(2, 1, 0, 3)  # [ch, p, d, c]
        # ztp layout: ch0dd0 | ch1dd0 | ch0dd1 | ch1dd1 | ch2 | ch3 | ch4
        ztp = np.concatenate(
            [
                zt_c[0][:, 0:2].reshape(128, 1024),
                zt_c[1][:, 0:2].reshape(128, 1024),
                zt_c[0][:, 2:4].reshape(128, 1024),
                zt_c[1][:, 2:4].reshape(128, 1024),
                zt_c[2].reshape(128, 2048),
                zt_c[3].reshape(128, 2048),
                zt_c[4].reshape(128, 2048),
            ],
            axis=1,
        )
        # xr/xo rows 512c..512c+511 packed [p, rr, 1024] then quartered along
        # the flattened free dim with io interleaved per quarter
        xr_c = xrec[CH * c : CH * (c + 1)].reshape(4, 128, 1024).transpose(1, 0, 2).reshape(128, 4096)
        xo_c = xorg[CH * c : CH * (c + 1)].reshape(4, 128, 1024).transpose(1, 0, 2).reshape(128, 4096)
        xin = np.stack(
            [xr_c.reshape(128, 4, 1024), xo_c.reshape(128, 4, 1024)], axis=2
        ).reshape(128, 8192)
        zi_c = repb[256 * c : 256 * (c + 1)].reshape(2, 128, D).transpose(1, 0, 2).reshape(128, 1024)
        zj_c = repb[2048 + 256 * c : 2048 + 256 * (c + 1)].reshape(2, 128, D).transpose(1, 0, 2).reshape(128, 1024)
        zin = np.concatenate([zi_c, zj_c], axis=1)
        in_maps.append(
            {
                "ztp": np.ascontiguousarray(ztp),
                "xin": np.ascontiguousarray(xin),
                "zin": np.ascontiguousarray(zin),
                "idf": np.eye(128, dtype=np.float32),
            }
        )
    return in_maps


def combine_outputs(results):
    """results: list of 8 dicts with out1a/out1b/ocol partials."""
    E2 = math.exp(1.0 / TAU)
    denom = np.zeros(TWO_N, dtype=np.float64)
    pos = np.zeros(TWO_N, dtype=np.float64)
    dxs = 0.0
    dzs = 0.0
    for c in range(NCORES):
        a = np.asarray(results[c]["out1a"], dtype=np.float64)  # [128, 12]
        b = np.asarray(results[c]["out1b"], dtype=np.float64)  # [128, 18]
        # partition p, row group rr -> global row 512c + 128rr + p
        rsum = a.reshape(128, 4, 3).sum(axis=2)  # [p, rr]
        denom[CH * c : CH * (c + 1)] += rsum.T.reshape(-1)
        pos[CH * c : CH * (c + 1)] = b[:, 0:4].T.reshape(-1)
        # colsum block (ch, blk) -> global columns of chunk c+1+ch
        for ch in range(3):
            m = (c + 1 + ch) % NCORES
            cs = b[:, 5 + 4 * ch : 5 + 4 * (ch + 1)]  # [128 m, 4 blk]
            denom[CH * m : CH * (m + 1)] += cs.T.reshape(-1)
        dxs += b[:, 17].sum()
        dzs += b[:, 4].sum()
    denom -= E2
    closs = (np.log(denom) - pos * QS).sum() / TWO_N
    recon = dxs / TWO_N
    zrec = dzs / N
    loss = recon + closs + zrec
    f = np.float32
    return (f(loss), f(closs), f(recon), f(zrec))


def kernel(representation, xrecon, xorig):
    from concourse.bass_utils import run_bass_kernel_spmd

    nc = _get_nc()
    in_maps = make_in_maps(representation, xrecon, xorig)
    res = run_bass_kernel_spmd(nc, in_maps, core_ids=list(range(NCORES)))
    return combine_outputs(res.results)
